# revision 17
# baseline (speedup 1.0000x reference)
"""DF11-compressed linear layer on 8 Trainium2 NeuronCores.

y = x @ W^T + bias, where W [4096, 4096] bf16 is decoded from DF11
compression (per-element exponent code -> lut_exp, packed sign+mantissa
byte).

The decode is a pure bit-reassembly, so it runs on the HOST (exact, in
numpy) and the device kernel degenerates to a memory-bound GEMM: each
core streams its [4096, 512] bf16 W^T shard (4 MB) from HBM straight
into 32 accumulating matmuls. HBM traffic, not compute, is the roofline.

Sharding (column-parallel): out_features split 8 ways; outputs are
concatenated on the host.

Per-core program (all scheduling insights from neuron-profile traces):
  1. Exactly 8 DMAs total so each maps to a distinct DMAHW lane sem (a
     9th DMA would wait on lane reuse): combined x^T+bias+ones tensor,
     6 weight chunks (small-big-small per queue, alternating the two
     HWDGE queues sync/scalar), and the output store. Fat chunks matter:
     the HWDGE sequencer pays ~600 ns per dma_start regardless of size.
  2. 8 warmup matmuls on a zeroed scratch tile run while the weight
     stream lands: they hold the PE busy so the HAM clock gate unthrottles
     (1.2 -> 2.4 GHz) before the real k-steps begin.
  3. TensorE accumulates y[16, 512] over 32 k-steps as chunks land, then
     adds bias via a 33rd K=1 matmul (ones stationary, bias moving).
  4. y[16, 512] f32 DMAs to HBM straight out of PSUM.
"""

import numpy as np
import ml_dtypes

import concourse.mybir as mybir
import concourse.tile as tile
from concourse import bacc
from concourse.bass_utils import run_bass_kernel_spmd

O = 4096           # out_features
I = 4096           # in_features
B = 16             # batch
N_CORES = 8
OS = O // N_CORES  # 512 out_features per core
P = 128
N_KT = I // P      # k-tiles (32)

# weight chunks in k-tiles, (size, queue): queue 0 = scalar (free while
# sync carries the combined x^T tensor), queue 1 = sync. Sizes keep the
# per-chunk completion semaphores ~2-3 us apart so the PE never idles
# long enough for the HAM clock gate to re-throttle, byte-balance the
# two queues (sync also carries comb), and keep the first and last
# chunks small (early PE start, short PE drain after the last sem).
CHUNKS = [(4, 0), (6, 1), (7, 0), (5, 1), (4, 1), (6, 0)]
OFFS = [sum(c for c, _ in CHUNKS[:i]) for i in range(len(CHUNKS))]
N_WARM = 10        # PE warmup matmuls: HAM un-throttles only after ~3.4 us
                   # of UNINTERRUPTED activity (10 x ~427 ns cold = 4.3 us)
N_GAPWARM = 1      # one filler matmul after each chunk's k-steps keeps the
                   # PE duty cycle high through the short DMA-wait gaps

XCOLS = N_KT * B   # 512: x^T region of the combined tile
CCOLS = XCOLS + OS + B  # + bias row + ones row = 1040


def _build_program():
    nc = bacc.Bacc("TRN2", target_bir_lowering=False, enable_partition_id=False)

    w_d = nc.dram_tensor("w", [P, N_KT, OS], mybir.dt.bfloat16, kind="ExternalInput")
    comb_d = nc.dram_tensor("comb", [P, CCOLS], mybir.dt.bfloat16, kind="ExternalInput")
    y_d = nc.dram_tensor("y", [B, OS], mybir.dt.float32, kind="ExternalOutput")

    with tile.TileContext(nc) as tc:
        with (
            tc.tile_pool(name="const", bufs=1) as cpool,
            tc.tile_pool(name="wt", bufs=1) as wtpool,
            tc.tile_pool(name="psum_y", bufs=1, space="PSUM") as psy,
        ):
            # PE warmup: no data dependencies beyond the memset, so the PE
            # spins (and un-throttles) while the DMA stream is in flight.
            wscr = cpool.tile([P, OS], mybir.dt.bfloat16)
            nc.gpsimd.memset(wscr[:], 0.0)
            warm_ps = psy.tile([B, OS], mybir.dt.float32, name="warm_ps")
            for i in range(N_WARM):
                nc.tensor.matmul(
                    warm_ps[:], wscr[:, 0:B], wscr[:],
                    start=True, stop=True, skip_group_check=True,
                )

            comb_sb = cpool.tile([P, CCOLS], mybir.dt.bfloat16)
            nc.sync.dma_start(comb_sb[:], comb_d[:])

            wts = []
            for ch, (o, (ln, q)) in enumerate(zip(OFFS, CHUNKS)):
                wt = wtpool.tile([P, ln, OS], mybir.dt.bfloat16,
                                 tag=f"w{ch}", name=f"w{ch}")
                eng = nc.scalar if q == 0 else nc.sync
                eng.dma_start(wt[:], w_d[:, o:o + ln, :])
                wts.append(wt)

            y_ps = psy.tile([B, OS], mybir.dt.float32, name="y_ps")
            for j in range(N_KT):
                ch = next(i for i in range(len(CHUNKS))
                          if OFFS[i] <= j < OFFS[i] + CHUNKS[i][0])
                nc.tensor.matmul(
                    y_ps[:], comb_sb[:, j * B:(j + 1) * B],
                    wts[ch][:, j - OFFS[ch], :],
                    start=(j == 0), stop=False,
                    skip_group_check=True,
                )
                if j + 1 < N_KT and j + 1 == OFFS[ch] + CHUNKS[ch][0]:
                    for _ in range(N_GAPWARM):
                        nc.tensor.matmul(
                            warm_ps[:], wscr[:, 0:B], wscr[:],
                            start=True, stop=True, skip_group_check=True,
                        )
            # bias: y += ones[1,16]^T @ bias[1,512] as a K=1 k-step
            nc.tensor.matmul(
                y_ps[:], comb_sb[0:1, XCOLS + OS:XCOLS + OS + B],
                comb_sb[0:1, XCOLS:XCOLS + OS],
                start=False, stop=True, skip_group_check=True,
            )
            y_sb = cpool.tile([B, OS], mybir.dt.float32)
            nc.scalar.copy(y_sb[:], y_ps[:])
            nc.sync.dma_start(y_d[:], y_sb[:])

    nc.compile()
    return nc


_NC_CACHE = None


def _get_program():
    global _NC_CACHE
    if _NC_CACHE is None:
        _NC_CACHE = _build_program()
    return _NC_CACHE


def kernel(x, exp_idx, sign_mant, lut_exp, bias, trace=False, tmpdir=None):
    x = np.asarray(x, dtype=np.float32)
    exp_idx = np.asarray(exp_idx, dtype=np.int32)
    sign_mant = np.asarray(sign_mant, dtype=np.int32)
    lut_exp = np.asarray(lut_exp, dtype=np.int32)
    bias = np.asarray(bias, dtype=np.float32)

    # Host-side DF11 decode (bit-exact vs the reference):
    # bits = sign(1) | biased exponent(8) | mantissa(7)
    exp = lut_exp[exp_idx]
    bits = ((sign_mant >> 7) << 15) | (exp << 7) | (sign_mant & 0x7F)
    # W^T in k-tile-major device layout: wdev[p, t, o] = W^T[t*128+p, o]
    wT = bits.astype(np.uint16).T.reshape(N_KT, P, O)

    # x^T tiled to [partition, k-tile, batch], flattened into the combined
    # tile next to the bias row and the ones row for the K=1 bias matmul.
    xT = x.astype(ml_dtypes.bfloat16).T.reshape(N_KT, P, B).transpose(1, 0, 2)
    comb_base = np.zeros((P, CCOLS), dtype=ml_dtypes.bfloat16)
    comb_base[:, :XCOLS] = xT.reshape(P, XCOLS)
    comb_base[0, XCOLS + OS:XCOLS + OS + B] = 1.0

    in_maps = []
    for c in range(N_CORES):
        sl = slice(c * OS, (c + 1) * OS)
        comb = comb_base.copy()
        comb[0, XCOLS:XCOLS + OS] = bias[sl].astype(ml_dtypes.bfloat16)
        in_maps.append({
            "w": np.ascontiguousarray(
                wT[:, :, sl].transpose(1, 0, 2)
            ).view(ml_dtypes.bfloat16),
            "comb": comb,
        })

    nc = _get_program()
    res = run_bass_kernel_spmd(
        nc, in_maps, core_ids=list(range(N_CORES)), trace=trace, tmpdir=tmpdir
    )
    y = np.concatenate([r["y"] for r in res.results], axis=1)
    if trace:
        kernel.last_results = res
    return y
